# revision 34
# baseline (speedup 1.0000x reference)
"""CRF loss kernel for Trainium2 (8 NeuronCores, batch-parallel).

loss = -sum_b [ log_num(b) - log_den(b) ]

Per-core shard: 8 sequences, t-major layout col = t*8 + b.

The forward-algorithm partition function is computed WITHOUT a serial
T-step scan.  Products of CRF transfer operators M_t = diag(x_t) E^T
mix directions at ~0.3/step (Birkhoff contraction of E=exp(0.1*N)), so
after DELTA warmup steps any positive seed is parallel to the true
state up to a scalar.  The sequence is cut into chunks; every chunk
runs an independent ones-seeded multiplicative scan starting DELTA
steps before its record region, and all chunks of a phase advance in
lockstep (one small matmul + one DVE mul per step).  Chunk-to-chunk
scale factors are recovered on the host purely from overlapping norm
records (both chunks traverse the same global step with mixed states;
the ratio of their recorded 1^T u norms is the relative scale).  A
constant per-step rescale c (folded into the transition block) keeps
values in bf16 range.

Device work: fp8 DoubleRow projection (W^T X), exp (ACT), chunk scans
(PE matmul vs eaug + DVE mul vs exp(logits)); the raw endsum/norm
records AND the exp(logits+b) buffer are DMA'd out.  Host recovers the
emit score as sum of ln(expx) at the gold tags (exp already folds in
the bias), does all length selection, the kappa chain, and the final
combine in float64.

Scheduling notes: engine queues are in-order and cross-engine waits
are completion-counter thresholds, so phase scan steps are emitted
round-robin, paced against the block stream with zero hot drains ahead
of the tail blocks' projections; input DMAs ride the ACT/HWDGE queue,
mid-stream record flushes the idle Pool/SWDGE queue (the tail phase's
on ACT, idle by then); big constant memsets run on Pool to keep DVE
free for scan muls; the device stops each phase one step early and the
host evaluates the final-step record functionals from the exported
state, shortening the post-DMA tail.
"""

import numpy as np
import ml_dtypes

import concourse.bacc as bacc
import concourse.tile as tile
from concourse import mybir
from concourse.bass_utils import run_bass_kernel_spmd

B, T, E, K = 64, 512, 2048, 32
NCORES = 8
BL = B // NCORES            # 8 sequences per core
R = T * BL                  # 4096 columns, col = t*BL + b
NE = E // 128               # 16 contraction chunks of 128
NE2 = NE // 2               # 8 DoubleRow chunks of 256
NRB = 8                     # 8 projection blocks of 64 timesteps (512 cols)
TB = T // NRB               # 64 timesteps per block

# phase geometry: (t0, nt, L, DELTA); records cover t in (t0, t0+nt]
PHASES = [(0, 128, 8, 3), (128, 128, 8, 3), (256, 128, 4, 2),
          (384, 128, 4, 2)]
# block after which each phase's inputs exist
PH_READY = [1, 3, 5, 7]
PH_LATE = [False, False, False, False]
# rr rounds to drain after each block's emission
DRAIN_AFTER = {2: 6, 3: 6, 4: 5, 5: 0, 6: 0}
PADT = 6                    # pad timesteps before t=0 in the expx buffer
LC = -(np.log(32.0) + 0.41)       # ln of per-step rescale c

# derived chunk table: list of (s, L, DELTA, NS) in global order
CHUNKS = []
PH_INFO = []   # (first_chunk, n_chunks, cols, NS, L, DELTA, t0, rec_off)
_rec_off = 0
for (t0_, nt_, L_, D_) in PHASES:
    PH_INFO.append((len(CHUNKS), nt_ // L_, (nt_ // L_) * BL, L_ + D_, L_,
                    D_, t0_, _rec_off))
    for _i in range(nt_ // L_):
        CHUNKS.append((t0_ + _i * L_ - D_, L_, D_, L_ + D_))
    _rec_off += (L_ + D_) * (nt_ // L_) * BL
RECW_TOTAL = _rec_off
EXQW = (PADT + T) * BL      # exported exp(logits) width

F32 = mybir.dt.float32
BF16 = mybir.dt.bfloat16
FP8 = mybir.dt.float8e4

TRACE = False
TRACE_KW = {}
LAST_RESULT = None

_prog_cache = {}


def _build_program():
    nc = bacc.Bacc("TRN2", target_bir_lowering=False, debug=False)

    xt = nc.dram_tensor("xt", [NRB, 128, NE * 512], FP8, kind="ExternalInput").ap()
    w = nc.dram_tensor("w", [128, NE * K], FP8, kind="ExternalInput").ap()
    eaug = nc.dram_tensor("eaug", [K, K + 2], BF16, kind="ExternalInput").ap()
    bias1 = nc.dram_tensor("bias1", [K, 1], F32, kind="ExternalInput").ap()
    a0 = nc.dram_tensor("a0", [K, BL], BF16, kind="ExternalInput").ap()
    cvec = nc.dram_tensor("cvec", [K + 2, 1], F32, kind="ExternalInput").ap()
    rec = nc.dram_tensor("rec", [K + 2, RECW_TOTAL], BF16,
                         kind="ExternalOutput").ap()
    exq = nc.dram_tensor("exq", [K, EXQW], BF16, kind="ExternalOutput").ap()

    Exp = mybir.ActivationFunctionType.Exp
    DR = mybir.MatmulPerfMode.DoubleRow
    EXW = (PADT + T + 1) * BL + 600   # slack for strided AP views

    with tile.TileContext(nc) as tc:
        with tc.tile_pool(name="const", bufs=1) as cp:
            # critical-path loads first: X block 0 + W gate everything
            xtp = cp.tile([128, NRB * NE * 512], FP8, tag="xtp")
            xtiles = [xtp[:, rb * NE * 512:(rb + 1) * NE * 512]
                      for rb in range(NRB)]

            def emit_dma_block(rb, split=1):
                if split == 1:
                    nc.scalar.dma_start(out=xtiles[rb], in_=xt[rb])
                    return
                # asymmetric 6:2 split: the trailing piece stays above the
                # HWDGE desc-gen floor (no stream bubble) while only two
                # projection matmuls wait on the stream's final bytes
                cut = 6 * 1024
                nc.scalar.dma_start(out=xtiles[rb][:, 0:cut],
                                    in_=xt[rb][:, 0:cut])
                nc.scalar.dma_start(out=xtiles[rb][:, cut:],
                                    in_=xt[rb][:, cut:])

            emit_dma_block(0)
            w_sb = cp.tile([128, NE * K], FP8, tag="w")
            nc.scalar.dma_start(out=w_sb, in_=w)
            emit_dma_block(1)

            eaug_sb = cp.tile([K, K + 2], BF16, tag="eaug")
            nc.scalar.dma_start(out=eaug_sb, in_=eaug)
            b1_sb = cp.tile([K, 1], F32, tag="b1")
            nc.scalar.dma_start(out=b1_sb, in_=bias1)
            a0_sb = cp.tile([K, BL], BF16, tag="a0")
            nc.scalar.dma_start(out=a0_sb, in_=a0)
            cv_sb = cp.tile([K + 2, 1], F32, tag="cvec")
            nc.scalar.dma_start(out=cv_sb, in_=cvec)

            # exp(logits) buffer, col (t + PADT)*BL + b; rows 32/33 = 1.0
            # (they ride through as the endsum/norm record rows), pads
            # (t <= 0, t = T, slack) = 1.0.  Big memsets on idle Pool.
            expx = cp.tile([K + 2, EXW], BF16, tag="expx")
            nc.gpsimd.memset(expx[K:K + 2, :], 1.0)
            nc.gpsimd.memset(expx[0:K, 0:(PADT + 1) * BL], 1.0)
            nc.gpsimd.memset(expx[0:K, (PADT + T) * BL:EXW], 1.0)

            # per-phase u history (col block sigma holds state after step
            # sigma; rows 32/33 hold the endsum/norm records of step sigma)
            uh = []
            for p, (_, _, colsp, nsp, _, _, _, _) in enumerate(PH_INFO):
                t_ = cp.tile([K + 2, nsp * colsp], BF16, tag=f"uh{p}")
                nc.vector.memset(t_[:, 0:colsp], 1.0)   # ones seeds
                uh.append(t_)

            with tc.tile_pool(name="pp", bufs=4, space="PSUM") as ppp, \
                 tc.tile_pool(name="ps", bufs=3, space="PSUM") as psp:

                def emit_block(rb):
                    # projection: 8 fp8 DoubleRow matmuls (256-contraction)
                    pp = ppp.tile([K, 512], F32, tag="pp", name=f"pp{rb}")
                    for e2 in range(NE2):
                        w_ap = w_sb[:, e2 * 2 * K:(e2 + 1) * 2 * K].rearrange(
                            "p (two k) -> p two k", two=2)
                        x_ap = xtiles[rb][:, e2 * 1024:(e2 + 1) * 1024] \
                            .rearrange("p (two n) -> p two n", two=2)
                        nc.tensor.matmul(pp, w_ap, x_ap,
                                         start=(e2 == 0), stop=(e2 == NE2 - 1),
                                         perf_mode=DR)
                    # exp(logits + b) -> expx
                    c0 = (PADT + rb * TB) * BL
                    nc.scalar.activation(expx[0:K, c0:c0 + 512], pp, Exp,
                                         bias=b1_sb)

                def emit_phase_step(p, sig):
                    _, _, colsp, nsp, L_, D_, t0_, ro = PH_INFO[p]
                    u = uh[p]
                    off = (t0_ - D_ + sig + PADT) * BL
                    span = (colsp // BL) * L_ * BL
                    exv = expx[0:K + 2, off:off + span].rearrange(
                        "p (c q) -> p c q", q=L_ * BL)[:, :, 0:BL]
                    uout = u[:, sig * colsp:(sig + 1) * colsp].rearrange(
                        "p (c b) -> p c b", b=BL)
                    if sig == 1:
                        # ones seeds: Eaug^T 1 is a constant column-sum
                        # vector, so step 1 is a single SBUF-only
                        # per-partition scale (no matmul, no PSUM access)
                        nc.vector.tensor_scalar_mul(uout, exv, cv_sb)
                    else:
                        ps = psp.tile([K + 2, colsp], F32, tag="ps",
                                      name=f"ps{p}_{sig}")
                        nc.tensor.matmul(
                            ps, eaug_sb,
                            u[0:K, (sig - 1) * colsp:sig * colsp],
                            start=True, stop=True)
                        nc.vector.tensor_mul(
                            uout, ps.rearrange("p (c b) -> p c b", b=BL),
                            exv)
                    if p == 0 and sig == D_:
                        # replace chunk 0's warming state with the true
                        # alpha_0 (host-computed)
                        nc.gpsimd.tensor_copy(
                            u[0:K, D_ * colsp:D_ * colsp + BL], a0_sb)
                    dmaq = (nc.scalar if p == len(PH_INFO) - 1
                            else nc.gpsimd)
                    last = p == len(PH_INFO) - 1
                    if sig == nsp - 3:
                        # early history flush: col blocks 0..NS-3
                        dmaq.dma_start(
                            out=rec[:, ro:ro + (sig + 1) * colsp],
                            in_=u[:, 0:(sig + 1) * colsp])
                    if last and sig == nsp - 2:
                        # last phase: flush NS-2 early so the terminal DMA
                        # carries only one column block
                        h0 = (nsp - 2) * colsp
                        dmaq.dma_start(
                            out=rec[:, ro + h0:ro + (nsp - 1) * colsp],
                            in_=u[:, h0:(nsp - 1) * colsp])
                    if sig == nsp - 1:
                        h0 = (nsp - (1 if last else 2)) * colsp
                        dmaq.dma_start(
                            out=rec[:, ro + h0:ro + nsp * colsp],
                            in_=u[:, h0:nsp * colsp])

                # ---- paced emission: block stream + rr phase drains -------
                pending = []        # [phase, next_sig]
                nextph = 0

                def drain(nrounds):
                    for _ in range(nrounds):
                        if not pending:
                            return
                        for ent in list(pending):
                            p, sig = ent
                            emit_phase_step(p, sig)
                            ent[1] += 1
                            if ent[1] > PH_INFO[p][3] - 1:
                                pending.remove(ent)

                for rb in range(NRB):
                    if rb + 2 < NRB:
                        emit_dma_block(rb + 2, split=2 if rb + 2 >= 6 else 1)
                    emit_block(rb)
                    if rb == NRB - 1:
                        # exp(logits) export: host recovers the emit score
                        # from ln(expx) at the gold tags
                        nc.scalar.dma_start(out=exq,
                                            in_=expx[0:K, 0:EXQW])
                    while (nextph < len(PH_INFO) and PH_READY[nextph] == rb
                           and not PH_LATE[nextph]):
                        pending.append([nextph, 1])
                        nextph += 1
                    drain(DRAIN_AFTER.get(rb, 0))
                    while nextph < len(PH_INFO) and PH_READY[nextph] == rb:
                        pending.append([nextph, 1])
                        nextph += 1
                drain(10 ** 6)

    nc.compile()
    return nc


def _host_scores(y, maskf, trans, start, end, lengths):
    """Index-only score terms, summed over all b: start + trans + end
    contributions to the joint likelihood (emit + bias come from ln(expx))."""
    y64 = y.astype(np.int64)
    s = start.astype(np.float64)[y64[:, 0]].sum()
    tr = (trans.astype(np.float64)[y64[:, :-1], y64[:, 1:]] * maskf[:, 1:]).sum()
    last = y64[np.arange(y64.shape[0]), lengths - 1]
    e = end.astype(np.float64)[last].sum()
    return s + tr + e


def kernel(X, y, mask, W, b, transitions, start_transitions, end_transitions):
    global LAST_RESULT
    X = np.asarray(X, dtype=np.float32)
    y = np.asarray(y, dtype=np.int32)
    mask = np.asarray(mask)
    W = np.asarray(W, dtype=np.float32)
    b_vec = np.asarray(b, dtype=np.float32)
    trans = np.asarray(transitions, dtype=np.float32)
    start = np.asarray(start_transitions, dtype=np.float32)
    end = np.asarray(end_transitions, dtype=np.float32)

    if "nc" not in _prog_cache:
        _prog_cache["nc"] = _build_program()
    nc = _prog_cache["nc"]

    bf16 = ml_dtypes.bfloat16
    fp8 = ml_dtypes.float8_e4m3

    # replicated params
    w_host = np.ascontiguousarray(
        W.reshape(NE, 128, K).transpose(1, 0, 2).reshape(128, NE * K)
    ).astype(fp8)
    eaug_host = np.ones((K, K + 2), dtype=np.float32)
    eaug_host[:, :K] = np.exp(trans) * np.exp(LC)
    eaug_host[:, K] = np.exp(end)
    eaug_host = eaug_host.astype(bf16)
    # column sums of the (bf16-quantized) eaug, as the device matmul would
    # produce from a ones state
    cvec_host = eaug_host.astype(np.float32).sum(axis=0).reshape(K + 2, 1).copy()
    bias1_host = b_vec.reshape(K, 1).copy()

    maskf = mask.astype(np.float64)
    lengths = maskf.sum(axis=1).astype(np.int64)  # [B]

    in_maps = []
    host_side = np.zeros(NCORES, dtype=np.float64)
    for cid in range(NCORES):
        bs = slice(cid * BL, (cid + 1) * BL)
        Xs = X[bs]                                   # [BL, T, E]
        # X^T, t-major: XT[e, t*BL+b] = X[b, t, e]; then block layout
        # xt[rb, p, e*512 + col] = XT[e*128+p, rb*512+col]
        XT = Xs.transpose(2, 1, 0).reshape(E, R)
        xt_host = np.ascontiguousarray(
            XT.reshape(NE, 128, NRB, 512).transpose(2, 1, 0, 3)
            .reshape(NRB, 128, NE * 512)
        ).astype(fp8)
        ys = y[bs]

        # true initial state alpha_0 = exp(x_0 W + b + start), fp64 on host
        lg0 = Xs[:, 0, :].astype(np.float64) @ W.astype(np.float64)
        a0_host = np.exp(lg0 + b_vec + start).T.astype(bf16).copy()  # [K, BL]

        host_side[cid] = _host_scores(ys, maskf[bs], trans, start, end,
                                      lengths[bs])

        in_maps.append({
            "xt": xt_host,
            "w": w_host,
            "eaug": eaug_host,
            "bias1": bias1_host,
            "a0": a0_host,
            "cvec": cvec_host,
        })

    res = run_bass_kernel_spmd(
        nc, in_maps, core_ids=list(range(NCORES)), trace=TRACE, **TRACE_KW
    )
    LAST_RESULT = res

    tt = np.arange(T)
    loss = 0.0
    for cid in range(NCORES):
        out = res.results[cid]
        recs = np.asarray(out["rec"]).astype(np.float64)
        exqv = np.asarray(out["exq"]).astype(np.float64)  # [K, EXQW]
        lens = lengths[cid * BL:(cid + 1) * BL]
        ys = y[cid * BL:(cid + 1) * BL]
        ms = maskf[cid * BL:(cid + 1) * BL]

        # emit + bias score: ln(exp(logits+b)) at gold tags
        emit_total = 0.0
        for bi in range(BL):
            v = exqv[ys[bi].astype(np.int64), (tt + PADT) * BL + bi]
            emit_total += (np.log(v) * ms[bi]).sum()

        # unpack u histories: per phase p, [K+2, NS*cols]; records for
        # sigma <= NS-1 live in rows 32/33 of col block sigma; the sigma=NS
        # functionals are computed here from the final state u(NS-1)
        erec, nrec = {}, {}
        expend = np.exp(end.astype(np.float64))
        for p, (g0, nch, colsp, nsp, L_, D_, t0_, ro) in enumerate(PH_INFO):
            blockr = recs[:, ro:ro + nsp * colsp].reshape(
                K + 2, nsp, nch, BL)
            for i in range(nch):
                for sig in range(1, nsp):
                    erec[(g0 + i, sig)] = blockr[K, sig, i]
                    nrec[(g0 + i, sig)] = blockr[K + 1, sig, i]
                ufin = blockr[0:K, nsp - 1, i]          # [K, BL]
                erec[(g0 + i, nsp)] = expend @ ufin
                nrec[(g0 + i, nsp)] = ufin.sum(axis=0)

        CG = len(CHUNKS)
        lnk = np.zeros((CG, BL))
        lnk[0] = CHUNKS[0][2] * LC
        for g in range(1, CG):
            s_p, L_p, D_p, NS_p = CHUNKS[g - 1]
            s_c, L_c, D_c, NS_c = CHUNKS[g]
            lnk[g] = (lnk[g - 1] + (s_p - s_c) * LC
                      + np.log(nrec[(g - 1, NS_p)])
                      - np.log(nrec[(g, D_c)]))

        ln_den = np.zeros(BL)
        for bi in range(BL):
            ln_ = int(lens[bi])
            # chunk whose record region (s+D, s+D+L] contains ln_
            g = max(gi for gi, (s_, L_, D_, NS_) in enumerate(CHUNKS)
                    if s_ + D_ < ln_ or gi == 0)
            s_g, L_, D_, NS_ = CHUNKS[g]
            sigma = ln_ - s_g
            ln_den[bi] = (np.log(erec[(g, sigma)][bi]) + lnk[g, bi]
                          - (sigma - 1) * LC)

        loss += host_side[cid] + emit_total - ln_den.sum()
    return np.float32(-loss)
